# revision 5
# baseline (speedup 1.0000x reference)
"""GraphSAGE layer (nn_GraphSAGE_34660386079338) as a Bass/Tile kernel on 8 trn2 cores.

Reference computation (all fp32):
    deg = adj.sum(axis=0) + 1                # column sums of adj [N]
    h   = x / deg[:, None]                   # row-scale
    h   = relu(h @ W.T + b)
    h   = h / (||h||_2(rows) + 1e-7)         # row L2 normalize
    h   = (h - mean(h, 0)) / sqrt(var(h, 0) + 1e-5) * gamma + beta   # batchnorm

Sharding: adj is COLUMN-sharded across the 8 cores (device k gets
adj[:, k*1024:(k+1)*1024]). Column j's sum is deg[j], and deg[j] scales row j
of x — so device k locally computes the degrees for exactly its own 1024 node
rows with no collective. x is row-sharded to match; W/b/gamma/beta replicated.
The only cross-device communication is a 2 KB AllReduce of the batchnorm
sum/sumsq statistics.

Device program highlights:
  - adj streams HBM->SBUF as fp32->bf16 cast DMAs (SWDGE); column sums run on
    the TensorEngine as ones-stationary bf16 matmuls at 1 cycle/row (fp32
    matmuls stream at 4 cycles/row and would be the bottleneck). PSUM
    accumulates in fp32, so precision loss is only the bf16 rounding of adj
    (~1e-5 relative on an 8192-element sum).
  - x@W.T (tiny) is computed concurrently with the adj stream: x and W are
    transposed on the PE (infeat must sit on partitions for the contraction),
    then 16 fp32 matmuls produce y0 = x @ W.T.
  - deg arrives in PSUM as [1, 1024] (free-axis layout); eight K=1 matmuls
    against a [1,1] ones tile transpose it into [128, 8] partition layout.
"""

import sys

if "/opt/trn_rl_repo" not in sys.path:
    sys.path.insert(0, "/opt/trn_rl_repo")

import numpy as np

import concourse.bass as bass
import concourse.bacc as bacc
import concourse.tile as tile
from concourse import mybir
from concourse.bass_utils import run_bass_kernel_spmd
from concourse.masks import make_identity

N_CORES = 8
N = 8192
D = 256  # infeat == outfeat
SHARD = N // N_CORES  # 1024 node rows / adj columns per core
NB = SHARD // 128  # 8 node blocks of 128
GT = 16  # adj stream megatiles per core
GROWS = N // GT  # 512 rows per megatile
GA = GROWS // 128  # 4 row-subblocks per megatile
EPS_NORM = 1e-7
EPS_BN = 1e-5

F32 = mybir.dt.float32
BF16 = mybir.dt.bfloat16


def _bcast_p(ap, parts=128):
    """Partition-broadcast a 1-row AP (stride-0 partition dim) for DMA."""
    free = [list(d) for d in ap.ap][-1:]
    return bass.AP(tensor=ap.tensor, offset=ap.offset, ap=[[0, parts]] + free)


def build_module(debug_outs=False):
    nc = bacc.Bacc("TRN2", target_bir_lowering=False, debug=False, num_devices=N_CORES)

    adj_ap = nc.dram_tensor("adj", [N, SHARD], F32, kind="ExternalInput").ap()
    x_ap = nc.dram_tensor("x", [SHARD, D], F32, kind="ExternalInput").ap()
    w_ap = nc.dram_tensor("W", [D, D], F32, kind="ExternalInput").ap()
    b_ap = nc.dram_tensor("b", [D], F32, kind="ExternalInput").ap()
    gamma_ap = nc.dram_tensor("gamma", [D], F32, kind="ExternalInput").ap()
    beta_ap = nc.dram_tensor("beta", [D], F32, kind="ExternalInput").ap()
    out_ap = nc.dram_tensor("out", [SHARD, D], F32, kind="ExternalOutput").ap()
    dbg = {}
    if debug_outs:
        dbg["rdeg"] = nc.dram_tensor("dbg_rdeg", [128, NB], F32, kind="ExternalOutput").ap()
        dbg["y0"] = nc.dram_tensor("dbg_y0", [SHARD, D], F32, kind="ExternalOutput").ap()
        dbg["hn"] = nc.dram_tensor("dbg_hn", [SHARD, D], F32, kind="ExternalOutput").ap()
        dbg["stats"] = nc.dram_tensor("dbg_stats", [1, 2 * D], F32, kind="ExternalOutput").ap()
        dbg["deg_f"] = nc.dram_tensor("dbg_deg_f", [1, SHARD], F32, kind="ExternalOutput").ap()

    # DRAM views: rows r = g*512 + a*128 + p
    adj_r = adj_ap.rearrange("(g a p) m -> g a p m", a=GA, p=128)
    x_r = x_ap.rearrange("(a p) m -> a p m", p=128)  # a = node block
    w_r = w_ap.rearrange("(a p) m -> a p m", p=128)  # a = outfeat block
    out_r = out_ap.rearrange("(a p) m -> a p m", p=128)

    with tile.TileContext(nc) as tc:
        with (
            tc.tile_pool(name="adj_pool", bufs=4) as adj_pool,
            tc.tile_pool(name="big", bufs=1) as big,
            tc.tile_pool(name="small", bufs=1) as small,
            tc.tile_pool(name="work", bufs=3) as work,
            tc.tile_pool(name="ptmp", bufs=3, space="PSUM") as ptmp,
            tc.tile_pool(name="pacc", bufs=1, space="PSUM") as pacc,
            tc.tile_pool(name="ptail", bufs=1, space="PSUM") as ptail,
            tc.tile_pool(name="dram", bufs=1, space="DRAM") as dram,
        ):
            # ---- constants / small loads -------------------------------
            ones_bf = small.tile([128, 1], BF16)
            nc.vector.memset(ones_bf, 1.0)
            ones_f32 = small.tile([128, 1], F32)
            nc.vector.memset(ones_f32, 1.0)
            one_1 = small.tile([1, 1], F32)
            nc.vector.memset(one_1, 1.0)
            identity = small.tile([128, 128], F32)
            make_identity(nc, identity)
            zero_col = small.tile([128, 1], F32)
            nc.vector.memset(zero_col, 0.0)
            epsbn_col = small.tile([128, 1], F32)
            nc.vector.memset(epsbn_col, EPS_BN)

            b_bc = small.tile([128, D], F32)
            nc.gpsimd.dma_start(out=b_bc, in_=_bcast_p(b_ap))
            gamma_bc = small.tile([128, D], F32)
            nc.gpsimd.dma_start(out=gamma_bc, in_=_bcast_p(gamma_ap))
            beta_bc = small.tile([128, D], F32)
            nc.gpsimd.dma_start(out=beta_bc, in_=_bcast_p(beta_ap))

            x_all = big.tile([128, NB, D], F32)
            nc.sync.dma_start(out=x_all, in_=x_r.transpose([1, 0, 2]))
            w_sb = big.tile([128, 2, D], F32)
            nc.sync.dma_start(out=w_sb, in_=w_r.transpose([1, 0, 2]))

            # ---- transposes: infeat onto partitions --------------------
            wT = big.tile([128, 2, D], F32)  # [infeat_p, infeat_chunk, outfeat]
            for a in range(2):  # outfeat block
                for i in range(2):  # infeat chunk
                    pst = ptmp.tile([128, 256], F32, tag="tmp")
                    nc.tensor.transpose(
                        pst[:, 0:128], w_sb[:, a, i * 128 : (i + 1) * 128], identity
                    )
                    nc.vector.tensor_copy(wT[:, i, a * 128 : (a + 1) * 128], pst[:, 0:128])

            xT = big.tile([128, 2, SHARD], F32)  # [infeat_p, infeat_chunk, node]
            for nb in range(NB):
                for i in range(2):
                    pst = ptmp.tile([128, 256], F32, tag="tmp")
                    nc.tensor.transpose(
                        pst[:, 0:128], x_all[:, nb, i * 128 : (i + 1) * 128], identity
                    )
                    nc.vector.tensor_copy(xT[:, i, nb * 128 : (nb + 1) * 128], pst[:, 0:128])

            # ---- y0 = x @ W.T  (node blocks on partitions) -------------
            y0 = big.tile([128, NB, D], F32)
            for nb in range(NB):
                psy = ptmp.tile([128, D], F32, tag="tmp")
                for i in range(2):
                    nc.tensor.matmul(
                        psy,
                        lhsT=xT[:, i, nb * 128 : (nb + 1) * 128],
                        rhs=wT[:, i, :],
                        start=(i == 0),
                        stop=(i == 1),
                    )
                nc.vector.tensor_copy(y0[:, nb, :], psy)

            # ---- stream adj, accumulate column sums on PE --------------
            ps_d0 = pacc.tile([1, 512], F32, tag="ps_d0")
            ps_d1 = pacc.tile([1, 512], F32, tag="ps_d1")
            for g in range(GT):
                at = adj_pool.tile([128, GA, 1024], BF16, tag="at")
                nc.gpsimd.dma_start(out=at, in_=adj_r[g].transpose([1, 0, 2]))
                for a in range(GA):
                    nc.tensor.matmul(
                        ps_d0,
                        lhsT=ones_bf,
                        rhs=at[:, a, 0:512],
                        start=(g == 0 and a == 0),
                        stop=(g == GT - 1 and a == GA - 1),
                    )
                    nc.tensor.matmul(
                        ps_d1,
                        lhsT=ones_bf,
                        rhs=at[:, a, 512:1024],
                        start=(g == 0 and a == 0),
                        stop=(g == GT - 1 and a == GA - 1),
                    )

            # ---- deg -> partition layout, rdeg = 1/(deg+1) -------------
            deg_f = small.tile([1, SHARD], F32)
            nc.vector.tensor_copy(deg_f[:, 0:512], ps_d0)
            nc.vector.tensor_copy(deg_f[:, 512:1024], ps_d1)
            ps_dp = ptail.tile([128, NB], F32, tag="ps_dp")
            for nb in range(NB):
                nc.tensor.matmul(
                    ps_dp[:, nb : nb + 1],
                    lhsT=deg_f[0:1, nb * 128 : (nb + 1) * 128],
                    rhs=one_1,
                    start=True,
                    stop=True,
                )
            rdeg = small.tile([128, NB], F32)
            nc.vector.tensor_scalar_add(rdeg, ps_dp, 1.0)
            nc.vector.reciprocal(rdeg, rdeg)

            # ---- per-block: scale, +b, relu, L2 norm, BN partials ------
            hn = big.tile([128, NB, D], F32)
            # separate tiles: interleaved accumulation groups must not share
            # a PSUM bank (start=True clears has_written bank-wide)
            ps_s = ptail.tile([1, D], F32, tag="ps_s")
            ps_q = ptail.tile([1, D], F32, tag="ps_q")
            for nb in range(NB):
                t = work.tile([128, D], F32, tag="t")
                nc.vector.tensor_scalar_mul(t, y0[:, nb, :], rdeg[:, nb : nb + 1])
                nc.vector.tensor_add(t, t, b_bc)
                h = work.tile([128, D], F32, tag="h")
                nc.scalar.activation(h, t, mybir.ActivationFunctionType.Relu, bias=zero_col)
                sq = work.tile([128, D], F32, tag="sq")
                ss = work.tile([128, 1], F32, tag="ss")
                nc.scalar.activation(
                    sq,
                    h,
                    mybir.ActivationFunctionType.Square,
                    bias=zero_col,
                    accum_out=ss,
                )
                rno = work.tile([128, 1], F32, tag="rno")
                nc.scalar.activation(rno, ss, mybir.ActivationFunctionType.Sqrt, bias=zero_col)
                nc.vector.tensor_scalar_add(rno, rno, EPS_NORM)
                nc.vector.reciprocal(rno, rno)
                nc.vector.tensor_scalar_mul(hn[:, nb, :], h, rno)
                sqn = work.tile([128, D], F32, tag="sqn")
                nc.vector.tensor_scalar(
                    out=sqn,
                    in0=sq,
                    scalar1=rno,
                    scalar2=rno,
                    op0=mybir.AluOpType.mult,
                    op1=mybir.AluOpType.mult,
                )
                nc.tensor.matmul(
                    ps_s,
                    lhsT=ones_f32,
                    rhs=hn[:, nb, :],
                    start=(nb == 0),
                    stop=(nb == NB - 1),
                )
                nc.tensor.matmul(
                    ps_q,
                    lhsT=ones_f32,
                    rhs=sqn,
                    start=(nb == 0),
                    stop=(nb == NB - 1),
                )

            # ---- AllReduce of BN stats (2 KB) --------------------------
            stats = small.tile([1, 2 * D], F32)
            nc.vector.tensor_copy(stats[:, 0:D], ps_s)
            nc.vector.tensor_copy(stats[:, D : 2 * D], ps_q)
            cc_in = dram.tile([1, 2 * D], F32)
            cc_out = dram.tile([1, 2 * D], F32, addr_space="Shared")
            nc.sync.dma_start(out=cc_in, in_=stats)
            nc.gpsimd.collective_compute(
                "AllReduce",
                mybir.AluOpType.add,
                replica_groups=[list(range(N_CORES))],
                ins=[cc_in[:]],
                outs=[cc_out[:]],
            )
            s_bc = small.tile([128, D], F32)
            nc.gpsimd.dma_start(out=s_bc, in_=_bcast_p(cc_out[0:1, 0:D]))
            q_bc = small.tile([128, D], F32)
            nc.gpsimd.dma_start(out=q_bc, in_=_bcast_p(cc_out[0:1, D : 2 * D]))

            # ---- BN affine: A = gamma*rstd, B = beta - mean*A ----------
            mean = small.tile([128, D], F32)
            nc.vector.tensor_scalar_mul(mean, s_bc, 1.0 / N)
            var = small.tile([128, D], F32)
            nc.vector.tensor_scalar_mul(var, q_bc, 1.0 / N)
            msq = small.tile([128, D], F32)
            nc.vector.tensor_mul(msq, mean, mean)
            nc.vector.tensor_sub(var, var, msq)
            sd = small.tile([128, D], F32)
            nc.scalar.activation(
                sd, var, mybir.ActivationFunctionType.Sqrt, bias=epsbn_col
            )
            rstd = small.tile([128, D], F32)
            nc.vector.reciprocal(rstd, sd)
            A = small.tile([128, D], F32)
            nc.vector.tensor_mul(A, gamma_bc, rstd)
            B = small.tile([128, D], F32)
            nc.vector.tensor_mul(B, mean, A)
            nc.vector.tensor_sub(B, beta_bc, B)

            if debug_outs:
                nc.sync.dma_start(out=dbg["rdeg"], in_=rdeg)
                nc.sync.dma_start(out=dbg["deg_f"], in_=deg_f)
                y0v = dbg["y0"].rearrange("(a p) m -> a p m", p=128)
                nc.sync.dma_start(out=y0v.transpose([1, 0, 2]), in_=y0)
                hnv = dbg["hn"].rearrange("(a p) m -> a p m", p=128)
                nc.sync.dma_start(out=hnv.transpose([1, 0, 2]), in_=hn)
                st2 = small.tile([1, 2 * D], F32)
                nc.gpsimd.dma_start(out=st2, in_=cc_out[:])
                nc.sync.dma_start(out=dbg["stats"], in_=st2)

            # ---- apply + store -----------------------------------------
            out_all = big.tile([128, NB, D], F32)
            for nb in range(NB):
                nc.vector.tensor_mul(out_all[:, nb, :], hn[:, nb, :], A)
                nc.vector.tensor_add(out_all[:, nb, :], out_all[:, nb, :], B)
            nc.sync.dma_start(out=out_r.transpose([1, 0, 2]), in_=out_all)

    nc.compile()
    return nc


_NC_CACHE = [None]


def _get_module(debug_outs=False):
    key = bool(debug_outs)
    if _NC_CACHE[0] is None or _NC_CACHE[0][0] != key:
        _NC_CACHE[0] = (key, build_module(debug_outs))
    return _NC_CACHE[0][1]


def _run(inputs, trace=False, trace_cores=None, debug_outs=False):
    adj = np.ascontiguousarray(np.asarray(inputs["adj"], dtype=np.float32))
    x = np.ascontiguousarray(np.asarray(inputs["x"], dtype=np.float32))
    W = np.ascontiguousarray(np.asarray(inputs["W"], dtype=np.float32))
    b = np.ascontiguousarray(np.asarray(inputs["b"], dtype=np.float32))
    gamma = np.ascontiguousarray(np.asarray(inputs["gamma"], dtype=np.float32))
    beta = np.ascontiguousarray(np.asarray(inputs["beta"], dtype=np.float32))

    in_maps = []
    for k in range(N_CORES):
        sl = slice(k * SHARD, (k + 1) * SHARD)
        in_maps.append(
            {
                "adj": np.ascontiguousarray(adj[:, sl]),
                "x": np.ascontiguousarray(x[sl]),
                "W": W,
                "b": b,
                "gamma": gamma,
                "beta": beta,
            }
        )

    nc = _get_module(debug_outs=debug_outs)
    res = run_bass_kernel_spmd(
        nc,
        in_maps,
        core_ids=list(range(N_CORES)),
        trace=trace,
        trace_cores=trace_cores,
    )
    out = np.concatenate([res.results[k]["out"] for k in range(N_CORES)], axis=0)
    return out, res


def kernel(**inputs) -> np.ndarray:
    out, _ = _run(inputs)
    return out


# revision 6
# speedup vs baseline: 1.1069x; 1.1069x over previous
"""GraphSAGE layer (nn_GraphSAGE_34660386079338) as a Bass/Tile kernel on 8 trn2 cores.

Reference computation (all fp32):
    deg = adj.sum(axis=0) + 1                # column sums of adj [N]
    h   = x / deg[:, None]                   # row-scale
    h   = relu(h @ W.T + b)
    h   = h / (||h||_2(rows) + 1e-7)         # row L2 normalize
    h   = (h - mean(h, 0)) / sqrt(var(h, 0) + 1e-5) * gamma + beta   # batchnorm

Sharding: adj is COLUMN-sharded across the 8 cores (device k gets
adj[:, k*1024:(k+1)*1024]). Column j's sum is deg[j], and deg[j] scales row j
of x — so device k locally computes the degrees for exactly its own 1024 node
rows with no collective. x is row-sharded to match; W/b/gamma/beta replicated.
The only cross-device communication is a 2 KB AllReduce of the batchnorm
sum/sumsq statistics.

Device program highlights:
  - adj streams HBM->SBUF as fp32->bf16 cast DMAs (SWDGE); column sums run on
    the TensorEngine as ones-stationary bf16 matmuls at 1 cycle/row (fp32
    matmuls stream at 4 cycles/row and would be the bottleneck). PSUM
    accumulates in fp32, so precision loss is only the bf16 rounding of adj
    (~1e-5 relative on an 8192-element sum).
  - x@W.T (tiny) is computed concurrently with the adj stream: x and W are
    transposed on the PE (infeat must sit on partitions for the contraction),
    then 16 fp32 matmuls produce y0 = x @ W.T.
  - deg arrives in PSUM as [1, 1024] (free-axis layout); eight K=1 matmuls
    against a [1,1] ones tile transpose it into [128, 8] partition layout.
"""

import sys

if "/opt/trn_rl_repo" not in sys.path:
    sys.path.insert(0, "/opt/trn_rl_repo")

import numpy as np

import concourse.bass as bass
import concourse.bacc as bacc
import concourse.tile as tile
from concourse import mybir
from concourse.bass_utils import run_bass_kernel_spmd
from concourse.masks import make_identity

N_CORES = 8
N = 8192
D = 256  # infeat == outfeat
SHARD = N // N_CORES  # 1024 node rows / adj columns per core
NB = SHARD // 128  # 8 node blocks of 128
GT = 16  # adj stream megatiles per core
GROWS = N // GT  # 512 rows per megatile
GA = GROWS // 128  # 4 row-subblocks per megatile
EPS_NORM = 1e-7
EPS_BN = 1e-5

F32 = mybir.dt.float32
BF16 = mybir.dt.bfloat16


def _bcast_p(ap, parts=128):
    """Partition-broadcast a 1-row AP (stride-0 partition dim) for DMA."""
    free = [list(d) for d in ap.ap][-1:]
    return bass.AP(tensor=ap.tensor, offset=ap.offset, ap=[[0, parts]] + free)


def build_module(debug_outs=False):
    nc = bacc.Bacc("TRN2", target_bir_lowering=False, debug=False, num_devices=N_CORES)

    adj_ap = nc.dram_tensor("adj", [N, SHARD], F32, kind="ExternalInput").ap()
    x_ap = nc.dram_tensor("x", [SHARD, D], F32, kind="ExternalInput").ap()
    w_ap = nc.dram_tensor("W", [D, D], F32, kind="ExternalInput").ap()
    b_ap = nc.dram_tensor("b", [D], F32, kind="ExternalInput").ap()
    gamma_ap = nc.dram_tensor("gamma", [D], F32, kind="ExternalInput").ap()
    beta_ap = nc.dram_tensor("beta", [D], F32, kind="ExternalInput").ap()
    out_ap = nc.dram_tensor("out", [SHARD, D], F32, kind="ExternalOutput").ap()
    dbg = {}
    if debug_outs:
        dbg["rdeg"] = nc.dram_tensor("dbg_rdeg", [128, NB], F32, kind="ExternalOutput").ap()
        dbg["y0"] = nc.dram_tensor("dbg_y0", [SHARD, D], F32, kind="ExternalOutput").ap()
        dbg["hn"] = nc.dram_tensor("dbg_hn", [SHARD, D], F32, kind="ExternalOutput").ap()
        dbg["stats"] = nc.dram_tensor("dbg_stats", [1, 2 * D], F32, kind="ExternalOutput").ap()
        dbg["deg_f"] = nc.dram_tensor("dbg_deg_f", [1, SHARD], F32, kind="ExternalOutput").ap()

    # DRAM views: rows r = g*512 + a*128 + p
    adj_r = adj_ap.rearrange("(g a p) m -> g a p m", a=GA, p=128)
    x_r = x_ap.rearrange("(a p) m -> a p m", p=128)  # a = node block
    w_r = w_ap.rearrange("(a p) m -> a p m", p=128)  # a = outfeat block
    out_r = out_ap.rearrange("(a p) m -> a p m", p=128)

    with tile.TileContext(nc) as tc:
        with (
            tc.tile_pool(name="adj_pool", bufs=4) as adj_pool,
            tc.tile_pool(name="big", bufs=1) as big,
            tc.tile_pool(name="small", bufs=1) as small,
            tc.tile_pool(name="work", bufs=3) as work,
            tc.tile_pool(name="ptmp", bufs=3, space="PSUM") as ptmp,
            tc.tile_pool(name="pacc", bufs=1, space="PSUM") as pacc,
            tc.tile_pool(name="ptail", bufs=1, space="PSUM") as ptail,
            tc.tile_pool(name="dram", bufs=1, space="DRAM") as dram,
        ):
            # ---- collective warmup: the first collective of an execution
            # pays a ~65us ncfw/ENCD setup cost in this environment; fire a
            # tiny dummy AllReduce immediately so it overlaps the adj stream
            # and the real stats AllReduce later only pays ~15us.
            dummy_sb = small.tile([1, 8], F32)
            nc.vector.memset(dummy_sb, 0.0)
            dummy_in = dram.tile([1, 8], F32)
            dummy_out = dram.tile([1, 8], F32, addr_space="Shared")
            nc.sync.dma_start(out=dummy_in, in_=dummy_sb)
            nc.gpsimd.collective_compute(
                "AllReduce",
                mybir.AluOpType.add,
                replica_groups=[list(range(N_CORES))],
                ins=[dummy_in[:]],
                outs=[dummy_out[:]],
            )

            # ---- constants / small loads -------------------------------
            ones_bf = small.tile([128, 1], BF16)
            nc.vector.memset(ones_bf, 1.0)
            ones_f32 = small.tile([128, 1], F32)
            nc.vector.memset(ones_f32, 1.0)
            one_1 = small.tile([1, 1], F32)
            nc.vector.memset(one_1, 1.0)
            identity = small.tile([128, 128], F32)
            make_identity(nc, identity)
            zero_col = small.tile([128, 1], F32)
            nc.vector.memset(zero_col, 0.0)
            epsbn_col = small.tile([128, 1], F32)
            nc.vector.memset(epsbn_col, EPS_BN)

            b_bc = small.tile([128, D], F32)
            nc.gpsimd.dma_start(out=b_bc, in_=_bcast_p(b_ap))
            x_all = big.tile([128, NB, D], F32)
            nc.sync.dma_start(out=x_all, in_=x_r.transpose([1, 0, 2]))
            w_sb = big.tile([128, 2, D], F32)
            nc.sync.dma_start(out=w_sb, in_=w_r.transpose([1, 0, 2]))

            # ---- transposes: infeat onto partitions --------------------
            wT = big.tile([128, 2, D], F32)  # [infeat_p, infeat_chunk, outfeat]
            for a in range(2):  # outfeat block
                for i in range(2):  # infeat chunk
                    pst = ptmp.tile([128, 256], F32, tag="tmp")
                    nc.tensor.transpose(
                        pst[:, 0:128], w_sb[:, a, i * 128 : (i + 1) * 128], identity
                    )
                    nc.vector.tensor_copy(wT[:, i, a * 128 : (a + 1) * 128], pst[:, 0:128])

            xT = big.tile([128, 2, SHARD], F32)  # [infeat_p, infeat_chunk, node]
            for nb in range(NB):
                for i in range(2):
                    pst = ptmp.tile([128, 256], F32, tag="tmp")
                    nc.tensor.transpose(
                        pst[:, 0:128], x_all[:, nb, i * 128 : (i + 1) * 128], identity
                    )
                    nc.vector.tensor_copy(xT[:, i, nb * 128 : (nb + 1) * 128], pst[:, 0:128])

            # ---- y0 = x @ W.T  (node blocks on partitions) -------------
            y0 = big.tile([128, NB, D], F32)
            for nb in range(NB):
                psy = ptmp.tile([128, D], F32, tag="tmp")
                for i in range(2):
                    nc.tensor.matmul(
                        psy,
                        lhsT=xT[:, i, nb * 128 : (nb + 1) * 128],
                        rhs=wT[:, i, :],
                        start=(i == 0),
                        stop=(i == 1),
                    )
                nc.vector.tensor_copy(y0[:, nb, :], psy)

            # ---- stream adj, accumulate column sums on PE --------------
            ps_d0 = pacc.tile([1, 512], F32, tag="ps_d0")
            ps_d1 = pacc.tile([1, 512], F32, tag="ps_d1")
            for g in range(GT):
                at = adj_pool.tile([128, GA, 1024], BF16, tag="at")
                nc.gpsimd.dma_start(out=at, in_=adj_r[g].transpose([1, 0, 2]))
                for a in range(GA):
                    nc.tensor.matmul(
                        ps_d0,
                        lhsT=ones_bf,
                        rhs=at[:, a, 0:512],
                        start=(g == 0 and a == 0),
                        stop=(g == GT - 1 and a == GA - 1),
                    )
                    nc.tensor.matmul(
                        ps_d1,
                        lhsT=ones_bf,
                        rhs=at[:, a, 512:1024],
                        start=(g == 0 and a == 0),
                        stop=(g == GT - 1 and a == GA - 1),
                    )

            # ---- deg -> partition layout, rdeg = 1/(deg+1) -------------
            deg_f = small.tile([1, SHARD], F32)
            nc.vector.tensor_copy(deg_f[:, 0:512], ps_d0)
            nc.vector.tensor_copy(deg_f[:, 512:1024], ps_d1)
            ps_dp = ptail.tile([128, NB], F32, tag="ps_dp")
            for nb in range(NB):
                nc.tensor.matmul(
                    ps_dp[:, nb : nb + 1],
                    lhsT=deg_f[0:1, nb * 128 : (nb + 1) * 128],
                    rhs=one_1,
                    start=True,
                    stop=True,
                )
            rdeg = small.tile([128, NB], F32)
            nc.vector.tensor_scalar_add(rdeg, ps_dp, 1.0)
            nc.vector.reciprocal(rdeg, rdeg)

            # ---- batched tail: scale, +b, relu, L2 norm, BN partials ---
            # one op over [128, NB*D] with stride-0 broadcast APs beats 8
            # per-block op chains (fewer instructions, engines pipeline)
            rdeg_b = rdeg.unsqueeze(2).broadcast_to([128, NB, D])
            b_b = b_bc.unsqueeze(1).broadcast_to([128, NB, D])
            t_all = big.tile([128, NB, D], F32)
            nc.vector.tensor_mul(t_all, y0, rdeg_b)
            nc.vector.tensor_add(t_all, t_all, b_b)
            h_all = big.tile([128, NB, D], F32)
            nc.scalar.activation(
                h_all, t_all, mybir.ActivationFunctionType.Relu, bias=zero_col
            )
            sq_all = big.tile([128, NB, D], F32)
            nc.vector.tensor_mul(sq_all, h_all, h_all)
            ss8 = small.tile([128, NB], F32)
            nc.vector.reduce_sum(ss8, sq_all, axis=mybir.AxisListType.X)
            no8 = small.tile([128, NB], F32)
            nc.scalar.activation(
                no8, ss8, mybir.ActivationFunctionType.Sqrt, bias=zero_col
            )
            nc.vector.tensor_scalar_add(no8, no8, EPS_NORM)
            nc.vector.reciprocal(no8, no8)
            rno2 = small.tile([128, NB], F32)
            nc.vector.tensor_mul(rno2, no8, no8)
            hn = big.tile([128, NB, D], F32)
            nc.vector.tensor_mul(hn, h_all, no8.unsqueeze(2).broadcast_to([128, NB, D]))
            sqn_all = big.tile([128, NB, D], F32)
            nc.vector.tensor_mul(
                sqn_all, sq_all, rno2.unsqueeze(2).broadcast_to([128, NB, D])
            )
            # separate tiles: interleaved accumulation groups must not share
            # a PSUM bank (start=True clears has_written bank-wide)
            ps_s = ptail.tile([1, D], F32, tag="ps_s")
            ps_q = ptail.tile([1, D], F32, tag="ps_q")
            for nb in range(NB):
                nc.tensor.matmul(
                    ps_s,
                    lhsT=ones_f32,
                    rhs=hn[:, nb, :],
                    start=(nb == 0),
                    stop=(nb == NB - 1),
                )
                nc.tensor.matmul(
                    ps_q,
                    lhsT=ones_f32,
                    rhs=sqn_all[:, nb, :],
                    start=(nb == 0),
                    stop=(nb == NB - 1),
                )

            # ---- AllReduce of BN stats (2 KB) --------------------------
            stats = small.tile([1, 2 * D], F32)
            nc.vector.tensor_copy(stats[:, 0:D], ps_s)
            nc.vector.tensor_copy(stats[:, D : 2 * D], ps_q)
            cc_in = dram.tile([1, 2 * D], F32)
            cc_out = dram.tile([1, 2 * D], F32, addr_space="Shared")
            nc.sync.dma_start(out=cc_in, in_=stats)
            nc.gpsimd.collective_compute(
                "AllReduce",
                mybir.AluOpType.add,
                replica_groups=[list(range(N_CORES))],
                ins=[cc_in[:]],
                outs=[cc_out[:]],
            )
            sqb = small.tile([128, 2 * D], F32)
            nc.gpsimd.dma_start(out=sqb, in_=_bcast_p(cc_out[:]))
            gamma_bc = small.tile([128, D], F32)
            nc.gpsimd.dma_start(out=gamma_bc, in_=_bcast_p(gamma_ap))
            beta_bc = small.tile([128, D], F32)
            nc.gpsimd.dma_start(out=beta_bc, in_=_bcast_p(beta_ap))

            # ---- BN affine: A = gamma*rstd, B = beta - mean*A ----------
            mean = small.tile([128, D], F32)
            nc.vector.tensor_scalar_mul(mean, sqb[:, 0:D], 1.0 / N)
            var = small.tile([128, D], F32)
            nc.vector.tensor_scalar_mul(var, sqb[:, D : 2 * D], 1.0 / N)
            msq = small.tile([128, D], F32)
            nc.vector.tensor_mul(msq, mean, mean)
            nc.vector.tensor_sub(var, var, msq)
            sd = small.tile([128, D], F32)
            nc.scalar.activation(
                sd, var, mybir.ActivationFunctionType.Sqrt, bias=epsbn_col
            )
            rstd = small.tile([128, D], F32)
            nc.vector.reciprocal(rstd, sd)
            A = small.tile([128, D], F32)
            nc.vector.tensor_mul(A, gamma_bc, rstd)
            B = small.tile([128, D], F32)
            nc.vector.tensor_mul(B, mean, A)
            nc.vector.tensor_sub(B, beta_bc, B)

            # ---- apply + store -----------------------------------------
            out_all = big.tile([128, NB, D], F32)
            A_b = A.unsqueeze(1).broadcast_to([128, NB, D])
            B_b = B.unsqueeze(1).broadcast_to([128, NB, D])
            nc.vector.tensor_mul(out_all, hn, A_b)
            nc.vector.tensor_add(out_all, out_all, B_b)
            nc.sync.dma_start(out=out_r.transpose([1, 0, 2]), in_=out_all)

    nc.compile()
    return nc


_NC_CACHE = [None]


def _get_module(debug_outs=False):
    key = bool(debug_outs)
    if _NC_CACHE[0] is None or _NC_CACHE[0][0] != key:
        _NC_CACHE[0] = (key, build_module(debug_outs))
    return _NC_CACHE[0][1]


def _run(inputs, trace=False, trace_cores=None, debug_outs=False):
    adj = np.ascontiguousarray(np.asarray(inputs["adj"], dtype=np.float32))
    x = np.ascontiguousarray(np.asarray(inputs["x"], dtype=np.float32))
    W = np.ascontiguousarray(np.asarray(inputs["W"], dtype=np.float32))
    b = np.ascontiguousarray(np.asarray(inputs["b"], dtype=np.float32))
    gamma = np.ascontiguousarray(np.asarray(inputs["gamma"], dtype=np.float32))
    beta = np.ascontiguousarray(np.asarray(inputs["beta"], dtype=np.float32))

    in_maps = []
    for k in range(N_CORES):
        sl = slice(k * SHARD, (k + 1) * SHARD)
        in_maps.append(
            {
                "adj": np.ascontiguousarray(adj[:, sl]),
                "x": np.ascontiguousarray(x[sl]),
                "W": W,
                "b": b,
                "gamma": gamma,
                "beta": beta,
            }
        )

    nc = _get_module(debug_outs=debug_outs)
    res = run_bass_kernel_spmd(
        nc,
        in_maps,
        core_ids=list(range(N_CORES)),
        trace=trace,
        trace_cores=trace_cores,
    )
    out = np.concatenate([res.results[k]["out"] for k in range(N_CORES)], axis=0)
    return out, res


def kernel(**inputs) -> np.ndarray:
    out, _ = _run(inputs)
    return out
